# revision 1
# baseline (speedup 1.0000x reference)
"""Causal multi-head self-attention (RoPE) Trainium2 kernel.

Model (from the reference nn.Module):
  D_MODEL=1024, NUM_HEADS=16, D_K=64, THETA=10000, BATCH=2, SEQ=2048.
  qkv = x @ w_qkv.T ; q,k get interleaved-pair RoPE; causal softmax(q k^T/8) v;
  out = attn_out @ w_o.T.

Sharding: tensor-parallel over heads. 8 cores x 2 heads each. x is
replicated (transposed on host), per-core w_qkv/w_o head slices. Each core
produces a partial y.T (full [1024, 4096]); host sums partials and
transposes back.

On-device layout is fully "transposed" (feature-on-partition, token-on-free):
  xT [1024, 4096], qT/kT [128, 4096] (2 heads x 64 dims on partitions),
  score tiles sT [k=128, q=512] for both heads side by side in one 2-bank
  PSUM tile, causal mask added on the PE (identity x (-30000) table matmul),
  one exp per k-chunk on ACT, PV against PE-transposed V with an appended
  ones column producing the softmax denominators in the same matmul,
  normalization by reciprocal + DMA partition-broadcast, final projection
  contracting the 128 on-core head-dims.

All matmul operands are float32r (TF32-class, full PE rate at N>=512).
"""

import math
import numpy as np
from contextlib import ExitStack

import concourse.bacc as bacc
import concourse.mybir as mybir
import concourse.tile as tile
from concourse.bass_utils import run_bass_kernel_spmd

f32 = mybir.dt.float32
f32r = mybir.dt.float32r
f16 = mybir.dt.float16

D = 1024          # d_model
H = 16            # total heads
DK = 64           # head dim
B = 2
S = 2048
T = B * S         # 4096 tokens
NCORES = 8
HPC = H // NCORES  # heads per core = 2
THETA = 10000.0
NEG = -30000.0     # causal-mask additive constant (exp underflows to 0)

TCH = 512          # token chunk (matmul N)
NTCH = T // TCH    # 8
KCH = 128          # key chunk (score-tile partitions)
DCH = 128          # d_model contraction chunk
NBLK = T // KCH    # 32

SWAP_MASK = [m ^ 1 for m in range(32)]  # adjacent-pair swap, per 32-quadrant

_PROGRAM = None


def _build_program():
    nc = bacc.Bacc("TRN2", target_bir_lowering=False, debug=False)

    xT = nc.dram_tensor("xT", [D, T], f16, kind="ExternalInput")
    wqkvT = nc.dram_tensor("wqkvT", [D, 3 * 128], f16, kind="ExternalInput")
    woT = nc.dram_tensor("woT", [128, D], f32r, kind="ExternalInput")
    crep = nc.dram_tensor("crep", [128, S], f16, kind="ExternalInput")
    ssign = nc.dram_tensor("ssign", [128, S], f16, kind="ExternalInput")
    maskneg = nc.dram_tensor("maskneg", [128, 896], f16, kind="ExternalInput")
    onesd = nc.dram_tensor("onesd", [128, 64], f32r, kind="ExternalInput")
    identr = nc.dram_tensor("identr", [128, 128], f16, kind="ExternalInput")
    yT = nc.dram_tensor("yT", [D, T], mybir.dt.bfloat16, kind="ExternalOutput")

    xT_r = xT.rearrange("(n p) t -> n p t", p=DCH)          # [8, 128, T]
    wq_r = wqkvT.rearrange("(n p) c -> p n c", p=DCH)       # [128, 8, 384]

    with tile.TileContext(nc) as tc:
        with ExitStack() as ctx:
            singles = ctx.enter_context(tc.tile_pool(name="singles", bufs=1))

            wq_sb = singles.tile([128, 8, 3 * 128], f16)
            crep_sb = singles.tile([128, S], f16)
            ssign_sb = singles.tile([128, S], f16)
            for h4 in range(4):
                sl = slice(h4 * (S // 4), (h4 + 1) * (S // 4))
                nc.gpsimd.dma_start(out=crep_sb[:, sl], in_=crep[:, sl])
                nc.gpsimd.dma_start(out=ssign_sb[:, sl], in_=ssign[:, sl])
            mask_sb = singles.tile([128, 896], f16)
            nc.gpsimd.dma_start(out=mask_sb, in_=maskneg[:, :])
            identr_sb = singles.tile([128, 128], f16)
            nc.gpsimd.dma_start(out=identr_sb, in_=identr[:, :])
            wo_sb = singles.tile([128, D], f32r)
            nc.gpsimd.dma_start(out=wo_sb, in_=woT[:, :])
            ones_sb = singles.tile([1, 64], f32r)
            nc.gpsimd.dma_start(out=ones_sb, in_=onesd[0:1, 0:64])

            qT = singles.tile([128, T], f32r)
            kT = singles.tile([128, T], f32r)
            # V in natural layout per 128-token block:
            # cols 0:64 = V_A, col 64 = ones, 65:129 = V_B, col 129 = ones.
            # Both heads' lhsT slices end with the ones column -> softmax
            # sums land in OT row 64, O in rows 0:64.
            vaug = singles.tile([128, NBLK, 130], f16)
            nc.gpsimd.dma_start(out=vaug[:, :, 64], in_=onesd[:, 0:NBLK])
            nc.gpsimd.dma_start(out=vaug[:, :, 129], in_=onesd[:, 32:32 + NBLK])
            ocatT = singles.tile([128, T], f32r)

            xpool = ctx.enter_context(tc.tile_pool(name="xc", bufs=3))
            rope = ctx.enter_context(tc.tile_pool(name="rope", bufs=3))
            eps_p = ctx.enter_context(tc.tile_pool(name="e", bufs=8))
            rp = ctx.enter_context(tc.tile_pool(name="r", bufs=4))
            yp = ctx.enter_context(tc.tile_pool(name="y", bufs=3))

            _mk_pools = {}

            def qkv_chunk(tch, xc=None):
                ps1 = _mk_pools["ps1"]
                pst = _mk_pools["pst"]
                t0 = tch * TCH
                s0 = t0 % S  # RoPE tables repeat per batch
                if xc is None:
                    xc = xpool.tile([128, 8, TCH], f16, tag="xc")
                    for dc in range(8):
                        nc.sync.dma_start(
                            out=xc[:, dc, :], in_=xT_r[dc, :, t0:t0 + TCH])
                for mb in range(3):  # q, k, v
                    if mb == 2:
                        # V directly in natural layout: x-chunk as the
                        # stationary operand, per 128-token block
                        for sub in range(TCH // KCH):  # 4 token blocks
                            blk = tch * 4 + sub
                            fo = sub * KCH
                            pv = pst.tile([128, KCH], f32, tag="pv")
                            for dc in range(8):
                                nc.tensor.matmul(
                                    pv, xc[:, dc, fo:fo + KCH],
                                    wq_sb[:, dc, 256:384],
                                    start=(dc == 0), stop=(dc == 7))
                            nc.scalar.activation(
                                out=vaug[:, blk, 0:64], in_=pv[:, 0:64],
                                func=mybir.ActivationFunctionType.Copy)
                            nc.vector.tensor_copy(
                                out=vaug[:, blk, 65:129], in_=pv[:, 64:128])
                        continue
                    ps = ps1.tile([128, TCH], f32, tag="qkvps")
                    for dc in range(8):
                        nc.tensor.matmul(
                            ps, wq_sb[:, dc, mb * 128:(mb + 1) * 128],
                            xc[:, dc, :],
                            start=(dc == 0), stop=(dc == 7))
                    if mb < 2:
                        dst = qT if mb == 0 else kT
                        sh = rope.tile([128, TCH], f32, tag="sh")
                        nc.vector.stream_shuffle(
                            out=sh, in_=ps, mask=SWAP_MASK)
                        tm1 = rope.tile([128, TCH], f32, tag="tm1")
                        nc.vector.tensor_tensor(
                            out=tm1, in0=ps, in1=crep_sb[:, s0:s0 + TCH],
                            op=mybir.AluOpType.mult)
                        tm2 = rope.tile([128, TCH], f32, tag="tm2")
                        nc.vector.tensor_tensor(
                            out=tm2, in0=sh, in1=ssign_sb[:, s0:s0 + TCH],
                            op=mybir.AluOpType.mult)
                        nc.vector.tensor_tensor(
                            out=dst[:, t0:t0 + TCH], in0=tm1, in1=tm2,
                            op=mybir.AluOpType.add)

            def attn_qi(b, qi):
                ps_s = _mk_pools["ss"]
                ps_ot = _mk_pools["ot"]
                toff = b * S
                boff = b * (S // KCH)
                q0 = toff + qi * TCH
                nkj = 4 * qi + 4
                otA = ps_ot.tile([65, TCH], f32, tag="ot")
                otB = ps_ot.tile([65, TCH], f32, tag="ot")
                for kj in range(nkj):
                    k0 = toff + kj * KCH
                    blk = boff + kj
                    # diagonal blocks: only columns [o, TCH) can be
                    # unmasked; skip the dead triangle region.
                    o = max(0, KCH * (kj - 4 * qi))
                    diag = kj >= 4 * qi
                    pAB = ps_s.tile([128, 2, TCH], f32, tag="sps")
                    nc.tensor.matmul(
                        pAB[:, 0, o:TCH], kT[0:64, k0:k0 + KCH],
                        qT[0:64, q0 + o:q0 + TCH],
                        start=True, stop=not diag, skip_group_check=True)
                    nc.tensor.matmul(
                        pAB[:, 1, o:TCH], kT[64:128, k0:k0 + KCH],
                        qT[64:128, q0 + o:q0 + TCH],
                        start=True, stop=not diag, skip_group_check=True)
                    if diag:  # additive causal mask via PE
                        msl = mask_sb[:, 384:896 - o]
                        nc.tensor.matmul(
                            pAB[:, 0, o:TCH], identr_sb, msl,
                            start=False, stop=True, skip_group_check=True)
                        nc.tensor.matmul(
                            pAB[:, 1, o:TCH], identr_sb, msl,
                            start=False, stop=True, skip_group_check=True)
                    eAB = eps_p.tile([128, 2, TCH], f16, tag="eT")
                    nc.scalar.activation(
                        out=eAB[:, :, o:TCH], in_=pAB[:, :, o:TCH],
                        func=mybir.ActivationFunctionType.Exp)
                    nc.tensor.matmul(
                        otA[:, o:TCH], vaug[:, blk, 0:65], eAB[:, 0, o:TCH],
                        start=(kj == 0), stop=(kj == nkj - 1),
                        skip_group_check=True)
                    nc.tensor.matmul(
                        otB[:, o:TCH], vaug[:, blk, 65:130], eAB[:, 1, o:TCH],
                        start=(kj == 0), stop=(kj == nkj - 1),
                        skip_group_check=True)
                # normalize: ocatT[:, q] = O_unnorm * (1/sums) broadcast.
                # DVE copies OT out of PSUM right away (frees the bank);
                # the rest runs SBUF-side on DVE/DMA/GPSIMD.
                for hi, otX in ((0, otA), (1, otB)):
                    ot_sb = rp.tile([65, TCH], f32, tag="otsb")
                    nc.vector.tensor_copy(out=ot_sb, in_=otX)
                    rX = rp.tile([1, TCH], f32r, tag="rr")
                    with nc.allow_low_precision(
                            reason="f32r softmax denominators"):
                        nc.vector.reciprocal(out=rX, in_=ot_sb[64:65, :])
                    bc_ps = ps_ot.tile([64, TCH], f32, tag="ot")
                    nc.tensor.matmul(bc_ps, ones_sb, rX,
                                     start=True, stop=True)
                    nc.vector.tensor_tensor(
                        out=ocatT[hi * 64:(hi + 1) * 64, q0:q0 + TCH],
                        in0=ot_sb[0:64, :], in1=bc_ps,
                        op=mybir.AluOpType.mult)

            def proj(b, half):
                ps_s = _mk_pools["ss"]
                toff = b * S
                if True:
                    h0 = toff + half * (S // 2)
                    for eb in range(8):  # output-embedding 128-blocks
                        pys = ps_s.tile([128, S // 2], f32, tag="sps")
                        for tq in range(2):
                            nc.tensor.matmul(
                                pys[:, tq * TCH:(tq + 1) * TCH],
                                wo_sb[:, eb * 128:(eb + 1) * 128],
                                ocatT[:, h0 + tq * TCH:h0 + (tq + 1) * TCH],
                                start=True, stop=True)
                        y_sb = yp.tile([128, S // 2], mybir.dt.bfloat16,
                                       tag="ysb")
                        if eb % 2 == 0:
                            nc.vector.tensor_copy(out=y_sb, in_=pys)
                        else:
                            nc.scalar.activation(
                                out=y_sb, in_=pys,
                                func=mybir.ActivationFunctionType.Copy)
                        nc.sync.dma_start(
                            out=yT[eb * 128:(eb + 1) * 128, h0:h0 + S // 2],
                            in_=y_sb)

            # ---- emission: QKV phase, then attention, then projection ---
            with ExitStack() as c1:
                ps1 = c1.enter_context(
                    tc.tile_pool(name="ps1", bufs=4, space="PSUM"))
                pst = c1.enter_context(
                    tc.tile_pool(name="pst", bufs=3, space="PSUM"))
                _mk_pools["ps1"] = ps1
                _mk_pools["pst"] = pst
                xc0 = xpool.tile([128, 8, TCH], f16, tag="xc")
                for dc in range(8):
                    nc.sync.dma_start(out=wq_sb[:, dc, :],
                                      in_=wq_r[:, dc, :])
                    nc.sync.dma_start(out=xc0[:, dc, :],
                                      in_=xT_r[dc, :, 0:TCH])
                for tch in range(NTCH):
                    qkv_chunk(tch, xc=xc0 if tch == 0 else None)
            with ExitStack() as c2:
                ps_s = c2.enter_context(
                    tc.tile_pool(name="ss", bufs=3, space="PSUM"))
                ps_ot = c2.enter_context(
                    tc.tile_pool(name="ot", bufs=2, space="PSUM"))
                _mk_pools["ss"] = ps_s
                _mk_pools["ot"] = ps_ot
                for qi in range(4):
                    attn_qi(0, qi)
                attn_qi(1, 0)
                proj(0, 0)
                proj(0, 1)
                attn_qi(1, 1)
                attn_qi(1, 2)
                attn_qi(1, 3)
                proj(1, 0)
                proj(1, 1)

    nc.compile()
    return nc


def _host_prep(x, token_positions, w_qkv, w_o):
    """Build per-core input maps."""
    x = np.asarray(x, dtype=np.float32)
    w_qkv = np.asarray(w_qkv, dtype=np.float32)
    w_o = np.asarray(w_o, dtype=np.float32)
    pos = np.asarray(token_positions).astype(np.float64)

    xT = np.ascontiguousarray(x.reshape(T, D).T).astype(np.float16)

    half = DK // 2
    inv_freq = THETA ** (-np.arange(half, dtype=np.float64) / half)  # [32]
    ang = pos[:, None] * inv_freq[None, :]          # [S, 32]
    cos = np.cos(ang).astype(np.float16)            # [S, 32]
    sin = np.sin(ang).astype(np.float16)

    # interleaved pair layout: partition p (within a head's 64) has freq p//2
    cos_rows = np.repeat(cos.T, 2, axis=0)          # [64, S]
    sin_rows = np.repeat(sin.T, 2, axis=0)
    sgn = np.where(np.arange(64) % 2 == 0, -1.0, 1.0).astype(np.float16)
    ssin_rows = sin_rows * sgn[:, None]
    crep = np.vstack([cos_rows, cos_rows])          # [128, 2048]
    ssign = np.vstack([ssin_rows, ssin_rows])

    jj = np.arange(896)[None, :]
    pp = np.arange(128)[:, None]
    maskneg = np.where(jj >= pp + 384, 0.0, NEG).astype(np.float16)

    onesd = np.ones((128, 64), dtype=np.float32)
    identr_np = np.eye(128, dtype=np.float16)

    scale = 1.0 / math.sqrt(DK)
    in_maps = []
    for c in range(NCORES):
        hA, hB = HPC * c, HPC * c + 1
        wq = np.empty((3 * 128, D), dtype=np.float32)
        wq[0:64] = w_qkv[hA * DK:(hA + 1) * DK] * scale
        wq[64:128] = w_qkv[hB * DK:(hB + 1) * DK] * scale
        wq[128:192] = w_qkv[D + hA * DK:D + (hA + 1) * DK]
        wq[192:256] = w_qkv[D + hB * DK:D + (hB + 1) * DK]
        wq[256:320] = w_qkv[2 * D + hA * DK:2 * D + (hA + 1) * DK]
        wq[320:384] = w_qkv[2 * D + hB * DK:2 * D + (hB + 1) * DK]
        wqkvT = np.ascontiguousarray(wq.T).astype(np.float16)

        woTc = np.ascontiguousarray(
            w_o[:, hA * DK:(hB + 1) * DK].T)        # [128, 1024]

        in_maps.append({
            "xT": xT, "wqkvT": wqkvT, "woT": woTc,
            "crep": crep, "ssign": ssign, "maskneg": maskneg,
            "onesd": onesd, "identr": identr_np,
        })
    return in_maps


def _get_program():
    global _PROGRAM
    if _PROGRAM is None:
        _PROGRAM = _build_program()
    return _PROGRAM


def run_sharded(in_maps, **kwargs):
    nc = _get_program()
    return run_bass_kernel_spmd(nc, in_maps, core_ids=list(range(NCORES)),
                                **kwargs)


def kernel(x, token_positions, w_qkv, w_o):
    in_maps = _host_prep(x, token_positions, w_qkv, w_o)
    res = run_sharded(in_maps)
    acc = np.zeros((D, T), dtype=np.float64)
    for c in range(NCORES):
        acc += res.results[c]["yT"].astype(np.float32)
    y = acc.T.astype(np.float32).reshape(B, S, D)
    return y



# revision 2
# speedup vs baseline: 1.2179x; 1.2179x over previous
"""Causal multi-head self-attention (RoPE) Trainium2 kernel.

Model (from the reference nn.Module):
  D_MODEL=1024, NUM_HEADS=16, D_K=64, THETA=10000, BATCH=2, SEQ=2048.
  qkv = x @ w_qkv.T ; q,k get interleaved-pair RoPE; causal softmax(q k^T/8) v;
  out = attn_out @ w_o.T.

Sharding: tensor-parallel over heads. 8 cores x 2 heads each. x is
replicated (transposed on host), per-core w_qkv/w_o head slices. Each core
produces a partial y.T (full [1024, 4096]); host sums partials and
transposes back.

Schedule: one merged software-pipelined stream. Attention segments
(b, qi) issue scores(kj) -> exp(kj) [ACT] with PV(kj-2) lagged two
iterations behind so the tensor engine never in-order-blocks on the exp.
QKV chunk production and the final w_o projection are "filler" work items
drained between attention iterations to keep PE busy during ACT-bound
stretches. Causal masking: one [128,128] triangle-mask matmul on the
first 128 columns of each diagonal block. Softmax denominators come from
an appended ones column in the PV stationary operand; normalization is
DVE reciprocal (from PSUM) + gpsimd partition_broadcast + DVE multiply.

PSUM: scores pool 2 tiles x 2 banks + ot pool 2 x 1 + filler 2 x 1 = 8.
"""

import math
from collections import deque
from functools import partial
import numpy as np
from contextlib import ExitStack

import concourse.bacc as bacc
import concourse.mybir as mybir
import concourse.tile as tile
from concourse.bass_utils import run_bass_kernel_spmd

f32 = mybir.dt.float32
f16 = mybir.dt.float16
bf16 = mybir.dt.bfloat16

D = 1024          # d_model
H = 16            # total heads
DK = 64           # head dim
B = 2
S = 2048
T = B * S         # 4096 tokens
NCORES = 8
HPC = H // NCORES  # heads per core = 2
THETA = 10000.0
NEG = -30000.0     # causal-mask additive constant (exp underflows to 0)

TCH = 512          # token chunk (matmul N)
NTCH = T // TCH    # 8
KCH = 128          # key chunk (score-tile partitions)
NBLK = T // KCH    # 32

SWAP_MASK = [m ^ 1 for m in range(32)]  # adjacent-pair swap, per 32-quadrant

_PROGRAM = None


def _build_program():
    nc = bacc.Bacc("TRN2", target_bir_lowering=False, debug=False)

    xT = nc.dram_tensor("xT", [D, T], f16, kind="ExternalInput")
    wqkvT = nc.dram_tensor("wqkvT", [D, 3 * 128], f16, kind="ExternalInput")
    woT = nc.dram_tensor("woT", [128, D], f16, kind="ExternalInput")
    crep = nc.dram_tensor("crep", [128, S], f16, kind="ExternalInput")
    ssign = nc.dram_tensor("ssign", [128, S], f16, kind="ExternalInput")
    masktri = nc.dram_tensor("masktri", [128, 128], f16, kind="ExternalInput")
    onesd = nc.dram_tensor("onesd", [128, 64], f16, kind="ExternalInput")
    identr = nc.dram_tensor("identr", [128, 128], f16, kind="ExternalInput")
    yT = nc.dram_tensor("yT", [D, T], bf16, kind="ExternalOutput")

    xT_r = xT.rearrange("(n p) t -> n p t", p=128)          # [8, 128, T]
    wq_r = wqkvT.rearrange("(n p) c -> p n c", p=128)       # [128, 8, 384]

    with tile.TileContext(nc) as tc:
        with ExitStack() as ctx:
            singles = ctx.enter_context(tc.tile_pool(name="singles", bufs=1))

            wq_sb = singles.tile([128, 8, 3 * 128], f16)
            crep_sb = singles.tile([128, S], f16)
            ssign_sb = singles.tile([128, S], f16)
            for h4 in range(4):
                sl = slice(h4 * (S // 4), (h4 + 1) * (S // 4))
                nc.gpsimd.dma_start(out=crep_sb[:, sl], in_=crep[:, sl])
                nc.gpsimd.dma_start(out=ssign_sb[:, sl], in_=ssign[:, sl])
            mask_sb = singles.tile([128, 128], f16)
            nc.gpsimd.dma_start(out=mask_sb, in_=masktri[:, :])
            identr_sb = singles.tile([128, 128], f16)
            nc.gpsimd.dma_start(out=identr_sb, in_=identr[:, :])
            wo_sb = singles.tile([128, D], f16)
            nc.gpsimd.dma_start(out=wo_sb, in_=woT[:, :])

            qT = singles.tile([128, T], f16)
            kT = singles.tile([128, T], f16)
            # V in natural layout per 128-token block:
            # cols 0:64 = V_A, col 64 = ones, 65:129 = V_B, col 129 = ones.
            vaug = singles.tile([128, NBLK, 130], f16)
            nc.gpsimd.dma_start(out=vaug[:, :, 64], in_=onesd[:, 0:NBLK])
            nc.gpsimd.dma_start(out=vaug[:, :, 129], in_=onesd[:, 32:32 + NBLK])
            ocatT = singles.tile([128, T], f16)

            xpool = ctx.enter_context(tc.tile_pool(name="xc", bufs=3))
            ropep = ctx.enter_context(tc.tile_pool(name="rope", bufs=3))
            eps_p = ctx.enter_context(tc.tile_pool(name="e", bufs=6))
            nrm = ctx.enter_context(tc.tile_pool(name="nrm", bufs=4))
            yp = ctx.enter_context(tc.tile_pool(name="y", bufs=4))

            ps_fill = ctx.enter_context(
                tc.tile_pool(name="pf", bufs=2, space="PSUM"))   # 2 banks
            ps_s = ctx.enter_context(
                tc.tile_pool(name="ss", bufs=2, space="PSUM"))   # 4 banks
            ps_ot = ctx.enter_context(
                tc.tile_pool(name="ot", bufs=2, space="PSUM"))   # 2 banks

            xc_tiles = {}

            # ---------- filler work items (qkv chunks + projection) -------
            filler = deque()
            drained = [0]

            def push(rows, fn):
                filler.append((rows, fn))

            def pop_one():
                rows, fn = filler.popleft()
                fn()
                drained[0] += 1
                return rows

            def drain_until(idx):
                while drained[0] <= idx:
                    pop_one()

            def drain_rows(budget):
                while filler and budget > 0:
                    budget -= pop_one()

            def drain_all():
                while filler:
                    pop_one()

            def emit_xdma(tch):
                xc = xpool.tile([128, 8, TCH], f16, tag="xc")
                t0 = tch * TCH
                for dc in range(8):
                    nc.sync.dma_start(out=xc[:, dc, :],
                                      in_=xT_r[dc, :, t0:t0 + TCH])
                xc_tiles[tch] = xc

            def emit_qk(tch, mb):   # mb 0=q, 1=k
                xc = xc_tiles[tch]
                t0 = tch * TCH
                s0 = t0 % S  # RoPE tables repeat per batch
                ps = ps_fill.tile([128, TCH], f32, tag="pf")
                for dc in range(8):
                    nc.tensor.matmul(
                        ps, wq_sb[:, dc, mb * 128:(mb + 1) * 128],
                        xc[:, dc, :], start=(dc == 0), stop=(dc == 7))
                dst = qT if mb == 0 else kT
                sh = ropep.tile([128, TCH], f32, tag="sh")
                nc.vector.stream_shuffle(out=sh, in_=ps, mask=SWAP_MASK)
                tm1 = ropep.tile([128, TCH], f32, tag="tm1")
                nc.vector.tensor_tensor(
                    out=tm1, in0=ps, in1=crep_sb[:, s0:s0 + TCH],
                    op=mybir.AluOpType.mult)
                tm2 = ropep.tile([128, TCH], f32, tag="tm2")
                nc.vector.tensor_tensor(
                    out=tm2, in0=sh, in1=ssign_sb[:, s0:s0 + TCH],
                    op=mybir.AluOpType.mult)
                nc.vector.tensor_tensor(
                    out=dst[:, t0:t0 + TCH], in0=tm1, in1=tm2,
                    op=mybir.AluOpType.add)

            def emit_v(tch):
                xc = xc_tiles[tch]
                pv = ps_fill.tile([128, 4, KCH], f32, tag="pf")
                for sub in range(4):
                    for dc in range(8):
                        nc.tensor.matmul(
                            pv[:, sub, :],
                            xc[:, dc, sub * KCH:(sub + 1) * KCH],
                            wq_sb[:, dc, 256:384],
                            start=(dc == 0), stop=(dc == 7),
                            skip_group_check=True)
                b0 = tch * 4
                nc.vector.tensor_copy(
                    out=vaug[:, b0:b0 + 4, 0:64], in_=pv[:, :, 0:64])
                nc.scalar.activation(
                    out=vaug[:, b0:b0 + 4, 65:129], in_=pv[:, :, 64:128],
                    func=mybir.ActivationFunctionType.Copy)

            _ycnt = [0]

            def emit_proj(b, half, eb, tq):
                h0 = b * S + half * (S // 2) + tq * TCH
                pys = ps_fill.tile([128, TCH], f32, tag="pf")
                nc.tensor.matmul(
                    pys, wo_sb[:, eb * 128:(eb + 1) * 128],
                    ocatT[:, h0:h0 + TCH], start=True, stop=True)
                y_sb = yp.tile([128, TCH], bf16, tag="ysb")
                eng = _ycnt[0] % 3
                _ycnt[0] += 1
                if eng == 0:
                    nc.vector.tensor_copy(out=y_sb, in_=pys)
                elif eng == 1:
                    nc.scalar.activation(
                        out=y_sb, in_=pys,
                        func=mybir.ActivationFunctionType.Copy)
                else:
                    nc.gpsimd.tensor_copy(out=y_sb, in_=pys)
                nc.sync.dma_start(
                    out=yT[eb * 128:(eb + 1) * 128, h0:h0 + TCH], in_=y_sb)

            # ---------- attention segment --------------------------------
            def attn_segment(b, qi):
                toff = b * S
                boff = b * (S // KCH)
                q0 = toff + qi * TCH
                nkj = 4 * qi + 4
                otA = ps_ot.tile([65, TCH], f32, tag="ot")
                otB = ps_ot.tile([65, TCH], f32, tag="ot")
                sc = {}
                ee = {}

                def scores(kj):
                    k0 = toff + kj * KCH
                    o = max(0, KCH * (kj - 4 * qi))
                    diag = kj >= 4 * qi
                    pAB = ps_s.tile([128, 2, TCH], f32, tag="sps")
                    nc.tensor.matmul(
                        pAB[:, 0, o:TCH], kT[0:64, k0:k0 + KCH],
                        qT[0:64, q0 + o:q0 + TCH],
                        start=True, stop=not diag, skip_group_check=True)
                    nc.tensor.matmul(
                        pAB[:, 1, o:TCH], kT[64:128, k0:k0 + KCH],
                        qT[64:128, q0 + o:q0 + TCH],
                        start=True, stop=not diag, skip_group_check=True)
                    if diag:  # triangle mask on first 128 cols only
                        nc.tensor.matmul(
                            pAB[:, 0, o:o + KCH], identr_sb, mask_sb,
                            start=False, stop=True, skip_group_check=True)
                        nc.tensor.matmul(
                            pAB[:, 1, o:o + KCH], identr_sb, mask_sb,
                            start=False, stop=True, skip_group_check=True)
                    sc[kj] = (pAB, o)

                def expf(kj):
                    pAB, o = sc.pop(kj)
                    eAB = eps_p.tile([128, 2, TCH], f16, tag="eT")
                    nc.scalar.activation(
                        out=eAB[:, :, o:TCH], in_=pAB[:, :, o:TCH],
                        func=mybir.ActivationFunctionType.Exp)
                    ee[kj] = (eAB, o)

                def pv(kj):
                    eAB, o = ee.pop(kj)
                    blk = boff + kj
                    nc.tensor.matmul(
                        otA[:, o:TCH], vaug[:, blk, 0:65], eAB[:, 0, o:TCH],
                        start=(kj == 0), stop=(kj == nkj - 1),
                        skip_group_check=True)
                    nc.tensor.matmul(
                        otB[:, o:TCH], vaug[:, blk, 65:130], eAB[:, 1, o:TCH],
                        start=(kj == 0), stop=(kj == nkj - 1),
                        skip_group_check=True)

                for kj in range(nkj):
                    scores(kj)
                    expf(kj)
                    if kj >= 2:
                        pv(kj - 2)
                    drain_rows(1200)
                pv(nkj - 2)
                pv(nkj - 1)

                for hi, otX in ((0, otA), (1, otB)):
                    rX = nrm.tile([1, TCH], f32, tag="rr")
                    nc.vector.reciprocal(out=rX, in_=otX[64:65, :])
                    bcX = nrm.tile([64, TCH], f32, tag="bc")
                    nc.gpsimd.partition_broadcast(bcX, rX, channels=64)
                    nc.vector.tensor_tensor(
                        out=ocatT[hi * 64:(hi + 1) * 64, q0:q0 + TCH],
                        in0=otX[0:64, :], in1=bcX,
                        op=mybir.AluOpType.mult)

            # ---------- emission -----------------------------------------
            for dc in range(8):
                nc.sync.dma_start(out=wq_sb[:, dc, :], in_=wq_r[:, dc, :])
            emit_xdma(0)
            emit_xdma(1)

            chunk_last = {}
            for tch in range(NTCH):
                if tch >= 2:
                    push(0, partial(emit_xdma, tch))
                push(4096, partial(emit_qk, tch, 0))
                push(4096, partial(emit_qk, tch, 1))
                push(4096, partial(emit_v, tch))
                chunk_last[tch] = len(filler) - 1

            # prologue: chunks 0 and 1 fully emitted before attention
            drain_until(chunk_last[1])

            for b in (0, 1):
                for qi in range(4):
                    drain_until(chunk_last[b * 4 + qi])
                    attn_segment(b, qi)
                for half in (0, 1):
                    for eb in range(8):
                        for tq in (0, 1):
                            push(512, partial(emit_proj, b, half, eb, tq))
            drain_all()

    nc.compile()
    return nc


def _host_prep(x, token_positions, w_qkv, w_o):
    """Build per-core input maps."""
    x = np.asarray(x, dtype=np.float32)
    w_qkv = np.asarray(w_qkv, dtype=np.float32)
    w_o = np.asarray(w_o, dtype=np.float32)
    pos = np.asarray(token_positions).astype(np.float64)

    xT = np.ascontiguousarray(x.reshape(T, D).T).astype(np.float16)

    half = DK // 2
    inv_freq = THETA ** (-np.arange(half, dtype=np.float64) / half)  # [32]
    ang = pos[:, None] * inv_freq[None, :]          # [S, 32]
    cos = np.cos(ang).astype(np.float16)            # [S, 32]
    sin = np.sin(ang).astype(np.float16)

    # interleaved pair layout: partition p (within a head's 64) has freq p//2
    cos_rows = np.repeat(cos.T, 2, axis=0)          # [64, S]
    sin_rows = np.repeat(sin.T, 2, axis=0)
    sgn = np.where(np.arange(64) % 2 == 0, -1.0, 1.0).astype(np.float16)
    ssin_rows = sin_rows * sgn[:, None]
    crep = np.vstack([cos_rows, cos_rows])          # [128, 2048]
    ssign = np.vstack([ssin_rows, ssin_rows])

    # triangle mask: col j of a diagonal 128-block is masked for key p > j
    jj = np.arange(128)[None, :]
    pp = np.arange(128)[:, None]
    masktri = np.where(jj >= pp, 0.0, NEG).astype(np.float16)

    onesd = np.ones((128, 64), dtype=np.float16)
    identr_np = np.eye(128, dtype=np.float16)

    scale = 1.0 / math.sqrt(DK)
    in_maps = []
    for c in range(NCORES):
        hA, hB = HPC * c, HPC * c + 1
        wq = np.empty((3 * 128, D), dtype=np.float32)
        wq[0:64] = w_qkv[hA * DK:(hA + 1) * DK] * scale
        wq[64:128] = w_qkv[hB * DK:(hB + 1) * DK] * scale
        wq[128:192] = w_qkv[D + hA * DK:D + (hA + 1) * DK]
        wq[192:256] = w_qkv[D + hB * DK:D + (hB + 1) * DK]
        wq[256:320] = w_qkv[2 * D + hA * DK:2 * D + (hA + 1) * DK]
        wq[320:384] = w_qkv[2 * D + hB * DK:2 * D + (hB + 1) * DK]
        wqkvT = np.ascontiguousarray(wq.T).astype(np.float16)

        woTc = np.ascontiguousarray(
            w_o[:, hA * DK:(hB + 1) * DK].T).astype(np.float16)  # [128, 1024]

        in_maps.append({
            "xT": xT, "wqkvT": wqkvT, "woT": woTc,
            "crep": crep, "ssign": ssign, "masktri": masktri,
            "onesd": onesd, "identr": identr_np,
        })
    return in_maps


def _get_program():
    global _PROGRAM
    if _PROGRAM is None:
        _PROGRAM = _build_program()
    return _PROGRAM


def run_sharded(in_maps, **kwargs):
    nc = _get_program()
    return run_bass_kernel_spmd(nc, in_maps, core_ids=list(range(NCORES)),
                                **kwargs)


def kernel(x, token_positions, w_qkv, w_o):
    in_maps = _host_prep(x, token_positions, w_qkv, w_o)
    res = run_sharded(in_maps)
    acc = np.zeros((D, T), dtype=np.float64)
    for c in range(NCORES):
        acc += res.results[c]["yT"].astype(np.float32)
    y = acc.T.astype(np.float32).reshape(B, S, D)
    return y
